# revision 24
# baseline (speedup 1.0000x reference)
"""STFT (DFT-as-conv) kernel for Trainium2, 8 NeuronCores.

Problem: x (16, 262144) f32, hann-windowed DFT kernels wsin/wcos
(2048, 1, 2048); reference reflect-pads by 1024, convolves with hop 512
-> returns (real, -imag), each (16, 2048, 513) f32.

Strategy (two symmetry folds on top of an im2col matmul):
  - Data-parallel over batch: 2 batches per core.
  - Hop-block im2col: n_fft = 4*hop, so frame matrices are shifted
    views of block-transposed copies of the padded signal.
  - Time-reversal fold: hann window is symmetric, W[k, 2048-n] =
    +/- W[k, n]; device folds frames into z = y[n] +/- y[2048-n],
    halving the contraction to 1024. win[0] = 0 kills the unpaired
    n=0 lane; sin(pi n) = 0 kills the sin n=1024 term; the cos n=1024
    column rides in the freed n=0 weight lane.
  - Bin-parity fold: W[1024-k, n] = (-1)^n W[k, n] (cos) and
    -(-1)^n W[k, n] (sin), so splitting contraction lanes by parity
    of n gives bins k and 1024-k from the same weight reads:
    E = even-lane partial sum, O = odd-lane partial sum;
    out[k] = E+O, out[1024-k] = +/-(E-O). The device ships raw E/O
    (plain PSUM->SBUF copies); host does the cheap +/-.
    Bin 1024 = E[0]-O[0] falls out free; bin 512 is a host matvec;
    bins 1025..2047 are host mirrors.
  - fp32r matmuls (full PE rate at even moving-dim >= 256). Frames
    padded 513 -> 514, split 258+256 (PSUM bank caps N at 512).
  - DMA shaped for the serialized-queue model: few large DMAs,
    weights on the scalar queue, column-split first transfers so the
    first matmul group waits on ~1.6 MB, not the whole input.
"""

import sys

sys.path.insert(0, "/opt/trn_rl_repo")

import numpy as np

BATCH = 16
LENGTH = 262144
N_FFT = 2048
HOP = 512
FRAMES = 513          # LENGTH // HOP + 1
PAD_FRAMES = 514      # frames padded to even for fp32r
BT_COLS = 520         # block columns padded so shifted views stay in range
N_GROUPS = ((0, 258), (258, 256))  # frame groups: start, size (even)
CORES = 8
B_PER_CORE = BATCH // CORES
N_UP = 8              # u' = kern*4 + mc, bins 0..511 in 4 chunks per kern
EXT = HOP * BT_COLS + 1537  # zero-extended xpad length for rev strides

_cache = {}


def _build_device_kernel():
    import concourse.bacc as bacc
    import concourse.mybir as mybir
    from concourse import tile

    nc = bacc.Bacc("TRN2", target_bir_lowering=False, debug=False,
                   num_devices=CORES)
    f32 = mybir.dt.float32
    f32r = mybir.dt.float32r

    # xin: 4 parity-packed signal arrays per batch:
    #   src 0: bte[jj,e,m] = xpad[512m + 256e + 2jj]
    #   src 1: rve[jj,e,m] = xpad[512m + 1536 - 256e - 2jj]
    #   src 2: bto[jj,e,m] = xpad[512m + 256e + 2jj + 1]
    #   src 3: rvo[jj,e,m] = xpad[512m + 1535 - 256e - 2jj]
    xin_d = nc.dram_tensor("xin", [B_PER_CORE, 4, 128, 2, BT_COLS], f32r,
                          kind="ExternalInput")
    # w[u', jj, par, c, mm]: folded parity weights for bins < 512
    w_d = nc.dram_tensor("w", [N_UP, 128, 2, 4, 128], f32r,
                         kind="ExternalInput")
    # o[u', mm, b*1028 + half*514 + f]: half 0 = E, 1 = O
    o_d = nc.dram_tensor("o", [N_UP, 128, B_PER_CORE * 2 * PAD_FRAMES],
                         f32, kind="ExternalOutput")

    with tile.TileContext(nc) as tc:
        with (
            tc.tile_pool(name="inp", bufs=1) as inp,
            tc.tile_pool(name="zp", bufs=1) as zpool,
            tc.tile_pool(name="wpool", bufs=8) as wpool,
            tc.tile_pool(name="op", bufs=4) as op,
            tc.tile_pool(name="psp", bufs=4, space="PSUM") as psp,
        ):
            ins = [[None] * 4 for _ in range(B_PER_CORE)]
            # z[par][s][b][c]: folded frames; par 0 = even, 1 = odd;
            # s 0 = plus (cos), 1 = minus (sin)
            zt = [[[[None] * 4 for _ in range(B_PER_CORE)]
                   for _ in range(2)] for _ in range(2)]
            for b in range(B_PER_CORE):
                for src in range(4):
                    ins[b][src] = inp.tile([128, 2, BT_COLS], f32r,
                                           name=f"in{b}{src}",
                                           tag=f"in{b}{src}")
                for par in range(2):
                    for s in range(2):
                        for c in range(4):
                            zt[par][s][b][c] = zpool.tile(
                                [128, PAD_FRAMES], f32r,
                                name=f"z{par}{s}{b}{c}",
                                tag=f"z{par}{s}{b}{c}")

            def fold(b, s, lo, hi):
                dve_op = (nc.vector.tensor_add, nc.vector.tensor_sub)[s]
                for par in range(2):
                    bt_t, rv_t = ins[b][2 * par], ins[b][2 * par + 1]
                    for c in range(4):
                        sh, rh = c // 2, 1 - c // 2
                        dve_op(out=zt[par][s][b][c][:, lo:hi],
                               in0=bt_t[:, c % 2, lo + sh:hi + sh],
                               in1=rv_t[:, c % 2, lo + rh:hi + rh])
                if s == 0:
                    # even lane (c=0, jj=0) is n=0: win[0] = 0 frees its
                    # weight slot for the cos n=1024 column; z+E lane 0
                    # must hold y_f[1024] = bte[0, 0, m+2].
                    nc.vector.tensor_copy(
                        out=zt[0][0][b][0][0:1, lo:hi],
                        in_=ins[b][0][0:1, 0, lo + 2:hi + 2])

            # Head scheduling: first matmul group needs w[0] + first-half
            # b0 inputs + the z+ folds of that half.
            SPLIT = 264
            MID = N_GROUPS[1][0]
            wts = []
            for up in range(N_UP):
                wts.append(wpool.tile([128, 2, 4, 128], f32r,
                                      name=f"wt{up}", tag="wt"))
            nc.scalar.dma_start(out=wts[0][:, 0], in_=w_d[0, :, 0])
            for src in range(2):
                nc.sync.dma_start(out=ins[0][src][:, :, :SPLIT],
                                  in_=xin_d[0, src, :, :, :SPLIT])
            nc.scalar.dma_start(out=wts[0][:, 1], in_=w_d[0, :, 1])
            for src in range(2, 4):
                nc.sync.dma_start(out=ins[0][src][:, :, :SPLIT],
                                  in_=xin_d[0, src, :, :, :SPLIT])
            fold(0, 0, 0, MID)
            nc.scalar.dma_start(out=wts[1], in_=w_d[1])
            for src in range(4):
                nc.sync.dma_start(out=ins[0][src][:, :, SPLIT:],
                                  in_=xin_d[0, src, :, :, SPLIT:])
            fold(0, 0, MID, PAD_FRAMES)
            for src in range(4):
                nc.sync.dma_start(out=ins[1][src][:, :, :SPLIT],
                                  in_=xin_d[1, src, :, :, :SPLIT])
            fold(1, 0, 0, MID)
            nc.scalar.dma_start(out=wts[2], in_=w_d[2])
            for src in range(4):
                nc.sync.dma_start(out=ins[1][src][:, :, SPLIT:],
                                  in_=xin_d[1, src, :, :, SPLIT:])
            fold(1, 0, MID, PAD_FRAMES)
            fold(0, 1, 0, PAD_FRAMES)
            fold(1, 1, 0, PAD_FRAMES)
            # remaining weights up front: the ACT queue must stay
            # DMA-only, or later weight loads block behind PSUM copies
            for up in range(3, N_UP):
                nc.scalar.dma_start(out=wts[up], in_=w_d[up])

            for up in range(N_UP):
                for b in range(B_PER_CORE):
                    kern = up // 4
                    wt = wts[up]
                    ot = op.tile([128, 2 * PAD_FRAMES], f32)
                    for f0, ng in N_GROUPS:
                        psE = psp.tile([128, ng], f32, name="psE",
                                       tag="psE")
                        psO = psp.tile([128, ng], f32, name="psO",
                                       tag="psO")
                        for c in range(4):
                            nc.tensor.matmul(
                                psE, wt[:, 0, c, :],
                                zt[0][kern][b][c][:, f0:f0 + ng],
                                start=(c == 0), stop=(c == 3))
                        for c in range(4):
                            nc.tensor.matmul(
                                psO, wt[:, 1, c, :],
                                zt[1][kern][b][c][:, f0:f0 + ng],
                                start=(c == 0), stop=(c == 3))
                        nc.vector.tensor_copy(
                            out=ot[:, f0:f0 + ng], in_=psE)
                        # ACT is otherwise idle; halves the DVE copy load
                        nc.scalar.copy(
                            out=ot[:, PAD_FRAMES + f0:PAD_FRAMES + f0 + ng],
                            in_=psO)
                    nc.sync.dma_start(
                        out=o_d[up, :, b * 2 * PAD_FRAMES:
                                (b + 1) * 2 * PAD_FRAMES],
                        in_=ot)
    nc.compile()
    return nc


def _get_nc():
    if "nc" not in _cache:
        _cache["nc"] = _build_device_kernel()
    return _cache["nc"]


def _host_prep(x, wsin, wcos):
    from numpy.lib.stride_tricks import as_strided

    x = np.asarray(x, dtype=np.float32)
    wsin = np.asarray(wsin, dtype=np.float32).reshape(N_FFT, N_FFT)
    wcos = np.asarray(wcos, dtype=np.float32).reshape(N_FFT, N_FFT)

    xpad = np.pad(x, ((0, 0), (N_FFT // 2, N_FFT // 2)), mode="reflect")
    xe = np.zeros((BATCH, EXT), np.float32)
    xe[:, :xpad.shape[1]] = xpad
    sb = xe.strides[1]
    s0 = xe.strides[0]

    xin = np.empty((BATCH, 4, 128, 2, BT_COLS), np.float32)
    shape = (BATCH, 128, 2, BT_COLS)
    xin[:, 0] = as_strided(xe, shape, (s0, 2 * sb, 256 * sb, 512 * sb))
    xin[:, 2] = as_strided(xe[:, 1:], shape,
                           (s0, 2 * sb, 256 * sb, 512 * sb))
    xin[:, 1] = as_strided(xe[:, 1536:], shape,
                           (s0, -2 * sb, -256 * sb, 512 * sb))
    xin[:, 3] = as_strided(xe[:, 1535:], shape,
                           (s0, -2 * sb, -256 * sb, 512 * sb))

    # folded parity weights for bin rows k < 512
    wf = np.empty((N_UP, 128, 2, 4, 128), np.float32)
    jj = np.arange(128)
    for kern, wm in enumerate((wcos, -wsin)):
        for mc in range(4):
            rows = wm[128 * mc:128 * mc + 128]       # (128 bins, 2048)
            for c in range(4):
                n_ev = 256 * c + 2 * jj
                wf[kern * 4 + mc, :, 0, c, :] = rows[:, n_ev].T
                wf[kern * 4 + mc, :, 1, c, :] = rows[:, n_ev + 1].T
    # n=0 even lane is dead (win[0] = 0): carry the cos n=1024 column
    wf[0:4, 0, 0, 0, :] = wcos[:512, 1024].reshape(4, 128)

    # host bin-512 rows (not representable in the parity fold)
    fr = np.lib.stride_tricks.sliding_window_view(
        xpad, N_FFT, axis=1)[:, ::HOP]               # (B, 513, 2048)
    row512 = np.empty((2, BATCH, FRAMES), np.float32)
    for kern, wm in enumerate((wcos, -wsin)):
        row512[kern] = np.einsum('bfn,n->bf', fr, wm[512],
                                 optimize=True).astype(np.float32)
    return xin, wf, row512


def _host_assemble(outs, row512):
    # outs: 8 arrays (8, 128, 2*2*514); E/O halves per batch
    per_batch_E, per_batch_O = [], []
    for o in outs:
        for b in range(B_PER_CORE):
            base = b * 2 * PAD_FRAMES
            per_batch_E.append(o[:, :, base:base + FRAMES])
            per_batch_O.append(
                o[:, :, base + PAD_FRAMES:base + PAD_FRAMES + FRAMES])
    E = np.stack(per_batch_E).reshape(BATCH, 2, 512, FRAMES)
    O = np.stack(per_batch_O).reshape(BATCH, 2, 512, FRAMES)

    outs_full = []
    for kern, msign in ((0, 1.0), (1, -1.0)):
        lo = E[:, kern] + O[:, kern]               # bins 0..511
        hi = E[:, kern] - O[:, kern]               # bins 1024-k
        if kern == 1:
            hi = -hi
        head = np.concatenate(
            [lo, row512[kern][:, None, :], hi[:, 511:0:-1], hi[:, 0:1]],
            axis=1)                                 # bins 0..1024
        full = np.concatenate([head, msign * head[:, 1023:0:-1]], axis=1)
        outs_full.append(np.ascontiguousarray(full, dtype=np.float32))
    return tuple(outs_full)


def kernel(x, wsin, wcos):
    from concourse.bass_utils import run_bass_kernel_spmd

    nc = _get_nc()
    xin, wf, row512 = _host_prep(x, wsin, wcos)
    in_maps = [
        {"xin": xin[i * B_PER_CORE:(i + 1) * B_PER_CORE], "w": wf}
        for i in range(CORES)
    ]
    res = run_bass_kernel_spmd(nc, in_maps, core_ids=list(range(CORES)))
    return _host_assemble(
        [res.results[i]["o"] for i in range(CORES)], row512)


# revision 35
# speedup vs baseline: 1.1370x; 1.1370x over previous
"""STFT (DFT-as-conv) kernel for Trainium2, 8 NeuronCores.

Problem: x (16, 262144) f32, hann-windowed DFT kernels wsin/wcos
(2048, 1, 2048); reference reflect-pads by 1024, convolves with hop 512
-> returns (real, -imag), each (16, 2048, 513) f32.

Strategy (two symmetry folds on top of an im2col matmul):
  - Data-parallel over batch: 2 batches per core.
  - Hop-block im2col: n_fft = 4*hop, so frame matrices are shifted
    views of block-transposed copies of the padded signal.
  - Time-reversal fold: hann window is symmetric, W[k, 2048-n] =
    +/- W[k, n]; device folds frames into z = y[n] +/- y[2048-n],
    halving the contraction to 1024. win[0] = 0 kills the unpaired
    n=0 lane; sin(pi n) = 0 kills the sin n=1024 term; the cos n=1024
    column rides in the freed n=0 weight lane.
  - Bin-parity fold: W[1024-k, n] = (-1)^n W[k, n] (cos) and
    -(-1)^n W[k, n] (sin), so splitting contraction lanes by parity
    of n gives bins k and 1024-k from the same weight reads:
    E = even-lane partial sum, O = odd-lane partial sum;
    out[k] = E+O, out[1024-k] = +/-(E-O). The device ships raw E/O
    (plain PSUM->SBUF copies); host does the cheap +/-.
    Bin 1024 = E[0]-O[0] falls out free; bin 512 is a host matvec;
    bins 1025..2047 are host mirrors.
  - fp32r matmuls (full PE rate at even moving-dim >= 256). Frames
    padded 513 -> 514, split 258+256 (PSUM bank caps N at 512).
  - DMA shaped for the serialized-queue model: few large DMAs,
    weights on the scalar queue, column-split first transfers so the
    first matmul group waits on ~1.6 MB, not the whole input.
"""

import sys

sys.path.insert(0, "/opt/trn_rl_repo")

import numpy as np

BATCH = 16
LENGTH = 262144
N_FFT = 2048
HOP = 512
FRAMES = 513          # LENGTH // HOP + 1
PAD_FRAMES = 514      # frames padded to even for fp32r
BT_COLS = 520         # block columns padded so shifted views stay in range
N_GROUPS = ((0, 258), (258, 256))  # frame groups: start, size (even)
CORES = 8
B_PER_CORE = BATCH // CORES
N_UP = 8              # u' = kern*4 + mc, bins 0..511 in 4 chunks per kern
EXT = HOP * BT_COLS + 1537  # zero-extended xpad length for rev strides

_cache = {}


def _build_device_kernel(whoist=False, ot_joint=True, obufs=4, psbufs=4,
                         out_eng="sync", in_eng="sync", w_eng="scalar",
                         out_split=True, **_ignored):
    import concourse.bacc as bacc
    import concourse.mybir as mybir
    from concourse import tile

    nc = bacc.Bacc("TRN2", target_bir_lowering=False, debug=False,
                   num_devices=CORES)
    f32 = mybir.dt.float32
    f32r = mybir.dt.float32r

    # xin: 4 parity-packed signal arrays per batch:
    #   src 0: bte[jj,e,m] = xpad[512m + 256e + 2jj]
    #   src 1: rve[jj,e,m] = xpad[512m + 1536 - 256e - 2jj]
    #   src 2: bto[jj,e,m] = xpad[512m + 256e + 2jj + 1]
    #   src 3: rvo[jj,e,m] = xpad[512m + 1535 - 256e - 2jj]
    xin_d = nc.dram_tensor("xin", [B_PER_CORE, 4, 128, 2, BT_COLS], f32r,
                          kind="ExternalInput")
    # w[u', jj, par, c, mm]: folded parity weights for bins < 512
    w_d = nc.dram_tensor("w", [N_UP, 128, 2, 4, 128], f32r,
                         kind="ExternalInput")
    # o[u', mm, b*1028 + half*514 + f]: half 0 = E, 1 = O
    o_d = nc.dram_tensor("o", [N_UP, 128, B_PER_CORE * 2 * PAD_FRAMES],
                         f32, kind="ExternalOutput")

    with tile.TileContext(nc) as tc:
        with (
            tc.tile_pool(name="inp", bufs=1) as inp,
            tc.tile_pool(name="zp", bufs=1) as zpool,
            tc.tile_pool(name="wpool", bufs=8) as wpool,
            tc.tile_pool(name="op", bufs=obufs) as op,
            tc.tile_pool(name="psp", bufs=psbufs, space="PSUM") as psp,
        ):
            ins = [[None] * 4 for _ in range(B_PER_CORE)]
            # z[par][s][b][c]: folded frames; par 0 = even, 1 = odd;
            # s 0 = plus (cos), 1 = minus (sin)
            zt = [[[[None] * 4 for _ in range(B_PER_CORE)]
                   for _ in range(2)] for _ in range(2)]
            for b in range(B_PER_CORE):
                for src in range(4):
                    ins[b][src] = inp.tile([128, 2, BT_COLS], f32r,
                                           name=f"in{b}{src}",
                                           tag=f"in{b}{src}")
                for par in range(2):
                    for s in range(2):
                        for c in range(4):
                            zt[par][s][b][c] = zpool.tile(
                                [128, PAD_FRAMES], f32r,
                                name=f"z{par}{s}{b}{c}",
                                tag=f"z{par}{s}{b}{c}")

            def fold(b, s, lo, hi):
                dve_op = (nc.vector.tensor_add, nc.vector.tensor_sub)[s]
                for par in range(2):
                    bt_t, rv_t = ins[b][2 * par], ins[b][2 * par + 1]
                    for c in range(4):
                        sh, rh = c // 2, 1 - c // 2
                        dve_op(out=zt[par][s][b][c][:, lo:hi],
                               in0=bt_t[:, c % 2, lo + sh:hi + sh],
                               in1=rv_t[:, c % 2, lo + rh:hi + rh])
                if s == 0:
                    # even lane (c=0, jj=0) is n=0: win[0] = 0 frees its
                    # weight slot for the cos n=1024 column; z+E lane 0
                    # must hold y_f[1024] = bte[0, 0, m+2].
                    nc.vector.tensor_copy(
                        out=zt[0][0][b][0][0:1, lo:hi],
                        in_=ins[b][0][0:1, 0, lo + 2:hi + 2])

            in_q = {"sync": nc.sync, "scalar": nc.scalar}[in_eng]
            w_q = {"sync": nc.sync, "scalar": nc.scalar}[w_eng]
            # Head scheduling: first matmul group needs w[0] + first-half
            # b0 inputs + the z+ folds of that half.
            SPLIT = 264
            MID = N_GROUPS[1][0]
            wts = []
            for up in range(N_UP):
                wts.append(wpool.tile([128, 2, 4, 128], f32r,
                                      name=f"wt{up}", tag="wt"))
            w_q.dma_start(out=wts[0][:, 0], in_=w_d[0, :, 0])
            for src in range(2):
                in_q.dma_start(out=ins[0][src][:, :, :SPLIT],
                                  in_=xin_d[0, src, :, :, :SPLIT])
            w_q.dma_start(out=wts[0][:, 1], in_=w_d[0, :, 1])
            for src in range(2, 4):
                in_q.dma_start(out=ins[0][src][:, :, :SPLIT],
                                  in_=xin_d[0, src, :, :, :SPLIT])
            fold(0, 0, 0, MID)
            w_q.dma_start(out=wts[1], in_=w_d[1])
            for src in range(4):
                in_q.dma_start(out=ins[0][src][:, :, SPLIT:],
                                  in_=xin_d[0, src, :, :, SPLIT:])
            fold(0, 0, MID, PAD_FRAMES)
            for src in range(4):
                in_q.dma_start(out=ins[1][src][:, :, :SPLIT],
                                  in_=xin_d[1, src, :, :, :SPLIT])
            fold(1, 0, 0, MID)
            w_q.dma_start(out=wts[2], in_=w_d[2])
            for src in range(4):
                in_q.dma_start(out=ins[1][src][:, :, SPLIT:],
                                  in_=xin_d[1, src, :, :, SPLIT:])
            fold(1, 0, MID, PAD_FRAMES)
            fold(0, 1, 0, PAD_FRAMES)
            fold(1, 1, 0, PAD_FRAMES)
            if whoist:
                # weights up front: keeps the ACT queue DMA-only
                for up in range(3, N_UP):
                    w_q.dma_start(out=wts[up], in_=w_d[up])

            for up in range(N_UP):
                kern = up // 4
                wt = wts[up]
                if not whoist and up >= 3:
                    w_q.dma_start(out=wt, in_=w_d[up])
                if ot_joint:
                    otj = op.tile([128, B_PER_CORE * 2 * PAD_FRAMES], f32,
                                  name="otj", tag="ot")
                for b in range(B_PER_CORE):
                    if ot_joint:
                        ot = otj[:, b * 2 * PAD_FRAMES:
                                 (b + 1) * 2 * PAD_FRAMES]
                    else:
                        ot = op.tile([128, 2 * PAD_FRAMES], f32,
                                     name="ot", tag="ot")
                    for f0, ng in N_GROUPS:
                        psE = psp.tile([128, ng], f32, name="psE",
                                       tag="psE")
                        psO = psp.tile([128, ng], f32, name="psO",
                                       tag="psO")
                        for c in range(4):
                            nc.tensor.matmul(
                                psE, wt[:, 0, c, :],
                                zt[0][kern][b][c][:, f0:f0 + ng],
                                start=(c == 0), stop=(c == 3))
                        for c in range(4):
                            nc.tensor.matmul(
                                psO, wt[:, 1, c, :],
                                zt[1][kern][b][c][:, f0:f0 + ng],
                                start=(c == 0), stop=(c == 3))
                        nc.vector.tensor_copy(
                            out=ot[:, f0:f0 + ng], in_=psE)
                        # ACT is otherwise idle; halves the DVE copy load
                        nc.scalar.copy(
                            out=ot[:, PAD_FRAMES + f0:PAD_FRAMES + f0 + ng],
                            in_=psO)
                    out_q = {"gpsimd": nc.gpsimd, "sync": nc.sync,
                             "scalar": nc.scalar}[out_eng]
                    base = b * 2 * PAD_FRAMES
                    if out_split:
                        out_q.dma_start(
                            out=o_d[up, :, base:base + PAD_FRAMES],
                            in_=ot[:, :PAD_FRAMES])
                        out_q.dma_start(
                            out=o_d[up, :, base + PAD_FRAMES:
                                    base + 2 * PAD_FRAMES],
                            in_=ot[:, PAD_FRAMES:])
                    else:
                        out_q.dma_start(
                            out=o_d[up, :, base:base + 2 * PAD_FRAMES],
                            in_=ot)
    nc.compile()
    return nc


def _get_nc():
    if "nc" not in _cache:
        _cache["nc"] = _build_device_kernel()
    return _cache["nc"]


def _host_prep(x, wsin, wcos):
    from numpy.lib.stride_tricks import as_strided

    x = np.asarray(x, dtype=np.float32)
    wsin = np.asarray(wsin, dtype=np.float32).reshape(N_FFT, N_FFT)
    wcos = np.asarray(wcos, dtype=np.float32).reshape(N_FFT, N_FFT)

    xpad = np.pad(x, ((0, 0), (N_FFT // 2, N_FFT // 2)), mode="reflect")
    xe = np.zeros((BATCH, EXT), np.float32)
    xe[:, :xpad.shape[1]] = xpad
    sb = xe.strides[1]
    s0 = xe.strides[0]

    xin = np.empty((BATCH, 4, 128, 2, BT_COLS), np.float32)
    shape = (BATCH, 128, 2, BT_COLS)
    xin[:, 0] = as_strided(xe, shape, (s0, 2 * sb, 256 * sb, 512 * sb))
    xin[:, 2] = as_strided(xe[:, 1:], shape,
                           (s0, 2 * sb, 256 * sb, 512 * sb))
    xin[:, 1] = as_strided(xe[:, 1536:], shape,
                           (s0, -2 * sb, -256 * sb, 512 * sb))
    xin[:, 3] = as_strided(xe[:, 1535:], shape,
                           (s0, -2 * sb, -256 * sb, 512 * sb))

    # folded parity weights for bin rows k < 512
    wf = np.empty((N_UP, 128, 2, 4, 128), np.float32)
    jj = np.arange(128)
    for kern, wm in enumerate((wcos, -wsin)):
        for mc in range(4):
            rows = wm[128 * mc:128 * mc + 128]       # (128 bins, 2048)
            for c in range(4):
                n_ev = 256 * c + 2 * jj
                wf[kern * 4 + mc, :, 0, c, :] = rows[:, n_ev].T
                wf[kern * 4 + mc, :, 1, c, :] = rows[:, n_ev + 1].T
    # n=0 even lane is dead (win[0] = 0): carry the cos n=1024 column
    wf[0:4, 0, 0, 0, :] = wcos[:512, 1024].reshape(4, 128)

    # host bin-512 rows (not representable in the parity fold)
    fr = np.lib.stride_tricks.sliding_window_view(
        xpad, N_FFT, axis=1)[:, ::HOP]               # (B, 513, 2048)
    row512 = np.empty((2, BATCH, FRAMES), np.float32)
    for kern, wm in enumerate((wcos, -wsin)):
        row512[kern] = np.einsum('bfn,n->bf', fr, wm[512],
                                 optimize=True).astype(np.float32)
    return xin, wf, row512


def _host_assemble(outs, row512):
    # outs: 8 arrays (8, 128, 2*2*514); E/O halves per batch
    per_batch_E, per_batch_O = [], []
    for o in outs:
        for b in range(B_PER_CORE):
            base = b * 2 * PAD_FRAMES
            per_batch_E.append(o[:, :, base:base + FRAMES])
            per_batch_O.append(
                o[:, :, base + PAD_FRAMES:base + PAD_FRAMES + FRAMES])
    E = np.stack(per_batch_E).reshape(BATCH, 2, 512, FRAMES)
    O = np.stack(per_batch_O).reshape(BATCH, 2, 512, FRAMES)

    outs_full = []
    for kern, msign in ((0, 1.0), (1, -1.0)):
        lo = E[:, kern] + O[:, kern]               # bins 0..511
        hi = E[:, kern] - O[:, kern]               # bins 1024-k
        if kern == 1:
            hi = -hi
        head = np.concatenate(
            [lo, row512[kern][:, None, :], hi[:, 511:0:-1], hi[:, 0:1]],
            axis=1)                                 # bins 0..1024
        full = np.concatenate([head, msign * head[:, 1023:0:-1]], axis=1)
        outs_full.append(np.ascontiguousarray(full, dtype=np.float32))
    return tuple(outs_full)


def kernel(x, wsin, wcos):
    from concourse.bass_utils import run_bass_kernel_spmd

    nc = _get_nc()
    xin, wf, row512 = _host_prep(x, wsin, wcos)
    in_maps = [
        {"xin": xin[i * B_PER_CORE:(i + 1) * B_PER_CORE], "w": wf}
        for i in range(CORES)
    ]
    res = run_bass_kernel_spmd(nc, in_maps, core_ids=list(range(CORES)))
    return _host_assemble(
        [res.results[i]["o"] for i in range(CORES)], row512)


# revision 36
# speedup vs baseline: 1.2540x; 1.1030x over previous
"""STFT (DFT-as-conv) kernel for Trainium2, 8 NeuronCores.

Problem: x (16, 262144) f32, hann-windowed DFT kernels wsin/wcos
(2048, 1, 2048); reference reflect-pads by 1024, convolves with hop 512
-> returns (real, -imag), each (16, 2048, 513) f32.

Strategy (two symmetry folds on top of an im2col matmul):
  - Data-parallel over batch: 2 batches per core.
  - Hop-block im2col: n_fft = 4*hop, so frame matrices are shifted
    views of block-transposed copies of the padded signal.
  - Time-reversal fold: hann window is symmetric, W[k, 2048-n] =
    +/- W[k, n]; device folds frames into z = y[n] +/- y[2048-n],
    halving the contraction to 1024. win[0] = 0 kills the unpaired
    n=0 lane; sin(pi n) = 0 kills the sin n=1024 term; the cos n=1024
    column rides in the freed n=0 weight lane.
  - Bin-parity fold: W[1024-k, n] = (-1)^n W[k, n] (cos) and
    -(-1)^n W[k, n] (sin), so splitting contraction lanes by parity
    of n gives bins k and 1024-k from the same weight reads:
    E = even-lane partial sum, O = odd-lane partial sum;
    out[k] = E+O, out[1024-k] = +/-(E-O). The device ships raw E/O
    (plain PSUM->SBUF copies); host does the cheap +/-.
    Bin 1024 = E[0]-O[0] falls out free; bin 512 is a host matvec;
    bins 1025..2047 are host mirrors.
  - fp32r matmuls (full PE rate at even moving-dim >= 256). Frames
    padded 513 -> 514, split 258+256 (PSUM bank caps N at 512).
  - DMA shaped for the serialized-queue model: few large DMAs,
    weights on the scalar queue, column-split first transfers so the
    first matmul group waits on ~1.6 MB, not the whole input.
"""

import sys

sys.path.insert(0, "/opt/trn_rl_repo")

import numpy as np

BATCH = 16
LENGTH = 262144
N_FFT = 2048
HOP = 512
FRAMES = 513          # LENGTH // HOP + 1
PAD_FRAMES = 514      # frames padded to even for fp32r
BT_COLS = 520         # block columns padded so shifted views stay in range
N_GROUPS = ((0, 258), (258, 256))  # frame groups: start, size (even)
CORES = 8
B_PER_CORE = BATCH // CORES
N_UP = 8              # u' = kern*4 + mc, bins 0..511 in 4 chunks per kern
EXT = HOP * BT_COLS + 1537  # zero-extended xpad length for rev strides

_cache = {}


def _build_device_kernel(whoist=False, ot_joint=True, obufs=4, psbufs=4,
                         out_eng="sync", in_eng="sync", w_eng="scalar",
                         out_split=True, **_ignored):
    import concourse.bacc as bacc
    import concourse.mybir as mybir
    from concourse import tile

    nc = bacc.Bacc("TRN2", target_bir_lowering=False, debug=False,
                   num_devices=CORES)
    f32 = mybir.dt.float32
    f32r = mybir.dt.float32r

    # xin: 4 parity-packed signal arrays per batch:
    #   src 0: bte[jj,e,m] = xpad[512m + 256e + 2jj]
    #   src 1: rve[jj,e,m] = xpad[512m + 1536 - 256e - 2jj]
    #   src 2: bto[jj,e,m] = xpad[512m + 256e + 2jj + 1]
    #   src 3: rvo[jj,e,m] = xpad[512m + 1535 - 256e - 2jj]
    xin_d = nc.dram_tensor("xin", [B_PER_CORE, 4, 128, 2, BT_COLS], f32r,
                          kind="ExternalInput")
    # w[u', jj, par, c, mm]: folded parity weights for bins < 512
    w_d = nc.dram_tensor("w", [N_UP, 128, 2, 4, 128], f32r,
                         kind="ExternalInput")
    # o[u', mm, b*1028 + half*514 + f]: half 0 = E, 1 = O
    o_d = nc.dram_tensor("o", [N_UP, 128, B_PER_CORE * 2 * PAD_FRAMES],
                         f32, kind="ExternalOutput")

    with tile.TileContext(nc) as tc:
        with (
            tc.tile_pool(name="inp", bufs=1) as inp,
            tc.tile_pool(name="zp", bufs=1) as zpool,
            tc.tile_pool(name="wpool", bufs=8) as wpool,
            tc.tile_pool(name="op", bufs=obufs) as op,
            tc.tile_pool(name="psp", bufs=psbufs, space="PSUM") as psp,
        ):
            ins = [[None] * 4 for _ in range(B_PER_CORE)]
            # z[par][s][b][c]: folded frames; par 0 = even, 1 = odd;
            # s 0 = plus (cos), 1 = minus (sin)
            zt = [[[[None] * 4 for _ in range(B_PER_CORE)]
                   for _ in range(2)] for _ in range(2)]
            for b in range(B_PER_CORE):
                for src in range(4):
                    ins[b][src] = inp.tile([128, 2, BT_COLS], f32r,
                                           name=f"in{b}{src}",
                                           tag=f"in{b}{src}")
                for par in range(2):
                    for s in range(2):
                        for c in range(4):
                            zt[par][s][b][c] = zpool.tile(
                                [128, PAD_FRAMES], f32r,
                                name=f"z{par}{s}{b}{c}",
                                tag=f"z{par}{s}{b}{c}")

            def fold(b, s, lo, hi):
                # sin-side folds ride the idle GpSimd engine so the DVE
                # stream (PSUM copies) never blocks behind them
                eng = nc.vector if s == 0 else nc.gpsimd
                dve_op = (eng.tensor_add, eng.tensor_sub)[s]
                for par in range(2):
                    bt_t, rv_t = ins[b][2 * par], ins[b][2 * par + 1]
                    for c in range(4):
                        sh, rh = c // 2, 1 - c // 2
                        dve_op(out=zt[par][s][b][c][:, lo:hi],
                               in0=bt_t[:, c % 2, lo + sh:hi + sh],
                               in1=rv_t[:, c % 2, lo + rh:hi + rh])
                if s == 0:
                    # even lane (c=0, jj=0) is n=0: win[0] = 0 frees its
                    # weight slot for the cos n=1024 column; z+E lane 0
                    # must hold y_f[1024] = bte[0, 0, m+2].
                    nc.vector.tensor_copy(
                        out=zt[0][0][b][0][0:1, lo:hi],
                        in_=ins[b][0][0:1, 0, lo + 2:hi + 2])

            in_q = {"sync": nc.sync, "scalar": nc.scalar}[in_eng]
            w_q = {"sync": nc.sync, "scalar": nc.scalar}[w_eng]
            # Head scheduling: first matmul group needs w[0] + first-half
            # b0 inputs + the z+ folds of that half.
            SPLIT = 264
            MID = N_GROUPS[1][0]
            wts = []
            for up in range(N_UP):
                wts.append(wpool.tile([128, 2, 4, 128], f32r,
                                      name=f"wt{up}", tag="wt"))
            w_q.dma_start(out=wts[0][:, 0], in_=w_d[0, :, 0])
            for src in range(2):
                in_q.dma_start(out=ins[0][src][:, :, :SPLIT],
                                  in_=xin_d[0, src, :, :, :SPLIT])
            w_q.dma_start(out=wts[0][:, 1], in_=w_d[0, :, 1])
            for src in range(2, 4):
                in_q.dma_start(out=ins[0][src][:, :, :SPLIT],
                                  in_=xin_d[0, src, :, :, :SPLIT])
            fold(0, 0, 0, MID)
            w_q.dma_start(out=wts[1], in_=w_d[1])
            for src in range(4):
                in_q.dma_start(out=ins[0][src][:, :, SPLIT:],
                                  in_=xin_d[0, src, :, :, SPLIT:])
            fold(0, 0, MID, PAD_FRAMES)
            for src in range(4):
                in_q.dma_start(out=ins[1][src][:, :, :SPLIT],
                                  in_=xin_d[1, src, :, :, :SPLIT])
            fold(1, 0, 0, MID)
            w_q.dma_start(out=wts[2], in_=w_d[2])
            for src in range(4):
                in_q.dma_start(out=ins[1][src][:, :, SPLIT:],
                                  in_=xin_d[1, src, :, :, SPLIT:])
            fold(1, 0, MID, PAD_FRAMES)
            fold(0, 1, 0, PAD_FRAMES)
            fold(1, 1, 0, PAD_FRAMES)
            if whoist:
                # weights up front: keeps the ACT queue DMA-only
                for up in range(3, N_UP):
                    w_q.dma_start(out=wts[up], in_=w_d[up])

            for up in range(N_UP):
                kern = up // 4
                wt = wts[up]
                if not whoist and up >= 3:
                    w_q.dma_start(out=wt, in_=w_d[up])
                if ot_joint:
                    otj = op.tile([128, B_PER_CORE * 2 * PAD_FRAMES], f32,
                                  name="otj", tag="ot")
                for b in range(B_PER_CORE):
                    if ot_joint:
                        ot = otj[:, b * 2 * PAD_FRAMES:
                                 (b + 1) * 2 * PAD_FRAMES]
                    else:
                        ot = op.tile([128, 2 * PAD_FRAMES], f32,
                                     name="ot", tag="ot")
                    for f0, ng in N_GROUPS:
                        psE = psp.tile([128, ng], f32, name="psE",
                                       tag="psE")
                        psO = psp.tile([128, ng], f32, name="psO",
                                       tag="psO")
                        for c in range(4):
                            nc.tensor.matmul(
                                psE, wt[:, 0, c, :],
                                zt[0][kern][b][c][:, f0:f0 + ng],
                                start=(c == 0), stop=(c == 3))
                        for c in range(4):
                            nc.tensor.matmul(
                                psO, wt[:, 1, c, :],
                                zt[1][kern][b][c][:, f0:f0 + ng],
                                start=(c == 0), stop=(c == 3))
                        nc.vector.tensor_copy(
                            out=ot[:, f0:f0 + ng], in_=psE)
                        # ACT is otherwise idle; halves the DVE copy load
                        nc.scalar.copy(
                            out=ot[:, PAD_FRAMES + f0:PAD_FRAMES + f0 + ng],
                            in_=psO)
                    out_q = {"gpsimd": nc.gpsimd, "sync": nc.sync,
                             "scalar": nc.scalar}[out_eng]
                    base = b * 2 * PAD_FRAMES
                    if out_split:
                        out_q.dma_start(
                            out=o_d[up, :, base:base + PAD_FRAMES],
                            in_=ot[:, :PAD_FRAMES])
                        out_q.dma_start(
                            out=o_d[up, :, base + PAD_FRAMES:
                                    base + 2 * PAD_FRAMES],
                            in_=ot[:, PAD_FRAMES:])
                    else:
                        out_q.dma_start(
                            out=o_d[up, :, base:base + 2 * PAD_FRAMES],
                            in_=ot)
    nc.compile()
    return nc


def _get_nc():
    if "nc" not in _cache:
        _cache["nc"] = _build_device_kernel()
    return _cache["nc"]


def _host_prep(x, wsin, wcos):
    from numpy.lib.stride_tricks import as_strided

    x = np.asarray(x, dtype=np.float32)
    wsin = np.asarray(wsin, dtype=np.float32).reshape(N_FFT, N_FFT)
    wcos = np.asarray(wcos, dtype=np.float32).reshape(N_FFT, N_FFT)

    xpad = np.pad(x, ((0, 0), (N_FFT // 2, N_FFT // 2)), mode="reflect")
    xe = np.zeros((BATCH, EXT), np.float32)
    xe[:, :xpad.shape[1]] = xpad
    sb = xe.strides[1]
    s0 = xe.strides[0]

    xin = np.empty((BATCH, 4, 128, 2, BT_COLS), np.float32)
    shape = (BATCH, 128, 2, BT_COLS)
    xin[:, 0] = as_strided(xe, shape, (s0, 2 * sb, 256 * sb, 512 * sb))
    xin[:, 2] = as_strided(xe[:, 1:], shape,
                           (s0, 2 * sb, 256 * sb, 512 * sb))
    xin[:, 1] = as_strided(xe[:, 1536:], shape,
                           (s0, -2 * sb, -256 * sb, 512 * sb))
    xin[:, 3] = as_strided(xe[:, 1535:], shape,
                           (s0, -2 * sb, -256 * sb, 512 * sb))

    # folded parity weights for bin rows k < 512
    wf = np.empty((N_UP, 128, 2, 4, 128), np.float32)
    jj = np.arange(128)
    for kern, wm in enumerate((wcos, -wsin)):
        for mc in range(4):
            rows = wm[128 * mc:128 * mc + 128]       # (128 bins, 2048)
            for c in range(4):
                n_ev = 256 * c + 2 * jj
                wf[kern * 4 + mc, :, 0, c, :] = rows[:, n_ev].T
                wf[kern * 4 + mc, :, 1, c, :] = rows[:, n_ev + 1].T
    # n=0 even lane is dead (win[0] = 0): carry the cos n=1024 column
    wf[0:4, 0, 0, 0, :] = wcos[:512, 1024].reshape(4, 128)

    # host bin-512 rows (not representable in the parity fold)
    fr = np.lib.stride_tricks.sliding_window_view(
        xpad, N_FFT, axis=1)[:, ::HOP]               # (B, 513, 2048)
    row512 = np.empty((2, BATCH, FRAMES), np.float32)
    for kern, wm in enumerate((wcos, -wsin)):
        row512[kern] = np.einsum('bfn,n->bf', fr, wm[512],
                                 optimize=True).astype(np.float32)
    return xin, wf, row512


def _host_assemble(outs, row512):
    # outs: 8 arrays (8, 128, 2*2*514); E/O halves per batch
    per_batch_E, per_batch_O = [], []
    for o in outs:
        for b in range(B_PER_CORE):
            base = b * 2 * PAD_FRAMES
            per_batch_E.append(o[:, :, base:base + FRAMES])
            per_batch_O.append(
                o[:, :, base + PAD_FRAMES:base + PAD_FRAMES + FRAMES])
    E = np.stack(per_batch_E).reshape(BATCH, 2, 512, FRAMES)
    O = np.stack(per_batch_O).reshape(BATCH, 2, 512, FRAMES)

    outs_full = []
    for kern, msign in ((0, 1.0), (1, -1.0)):
        lo = E[:, kern] + O[:, kern]               # bins 0..511
        hi = E[:, kern] - O[:, kern]               # bins 1024-k
        if kern == 1:
            hi = -hi
        head = np.concatenate(
            [lo, row512[kern][:, None, :], hi[:, 511:0:-1], hi[:, 0:1]],
            axis=1)                                 # bins 0..1024
        full = np.concatenate([head, msign * head[:, 1023:0:-1]], axis=1)
        outs_full.append(np.ascontiguousarray(full, dtype=np.float32))
    return tuple(outs_full)


def kernel(x, wsin, wcos):
    from concourse.bass_utils import run_bass_kernel_spmd

    nc = _get_nc()
    xin, wf, row512 = _host_prep(x, wsin, wcos)
    in_maps = [
        {"xin": xin[i * B_PER_CORE:(i + 1) * B_PER_CORE], "w": wf}
        for i in range(CORES)
    ]
    res = run_bass_kernel_spmd(nc, in_maps, core_ids=list(range(CORES)))
    return _host_assemble(
        [res.results[i]["o"] for i in range(CORES)], row512)


# revision 40
# speedup vs baseline: 1.3267x; 1.0579x over previous
"""STFT (DFT-as-conv) kernel for Trainium2, 8 NeuronCores.

Problem: x (16, 262144) f32, hann-windowed DFT kernels wsin/wcos
(2048, 1, 2048); reference reflect-pads by 1024, convolves with hop 512
-> returns (real, -imag), each (16, 2048, 513) f32.

Strategy (two symmetry folds on top of an im2col matmul):
  - Data-parallel over batch: 2 batches per core.
  - Hop-block im2col: n_fft = 4*hop, so frame matrices are shifted
    views of block-transposed copies of the padded signal.
  - Time-reversal fold: hann window is symmetric, W[k, 2048-n] =
    +/- W[k, n]; device folds frames into z = y[n] +/- y[2048-n],
    halving the contraction to 1024. win[0] = 0 kills the unpaired
    n=0 lane; sin(pi n) = 0 kills the sin n=1024 term; the cos n=1024
    column rides in the freed n=0 weight lane.
  - Bin-parity fold: W[1024-k, n] = (-1)^n W[k, n] (cos) and
    -(-1)^n W[k, n] (sin), so splitting contraction lanes by parity
    of n gives bins k and 1024-k from the same weight reads:
    E = even-lane partial sum, O = odd-lane partial sum;
    out[k] = E+O, out[1024-k] = +/-(E-O). The device ships raw E/O
    (plain PSUM->SBUF copies); host does the cheap +/-.
    Bin 1024 = E[0]-O[0] falls out free; bin 512 is a host matvec;
    bins 1025..2047 are host mirrors.
  - fp32r matmuls (full PE rate at even moving-dim >= 256). Frames
    padded 513 -> 514, split 258+256 (PSUM bank caps N at 512).
  - DMA shaped for the serialized-queue model: few large DMAs,
    weights on the scalar queue, column-split first transfers so the
    first matmul group waits on ~1.6 MB, not the whole input.
"""

import sys

sys.path.insert(0, "/opt/trn_rl_repo")

import numpy as np

BATCH = 16
LENGTH = 262144
N_FFT = 2048
HOP = 512
FRAMES = 513          # LENGTH // HOP + 1
PAD_FRAMES = 514      # frames padded to even for fp32r
BT_COLS = 520         # block columns padded so shifted views stay in range
N_GROUPS = ((0, 258), (258, 256))  # frame groups: start, size (even)
CORES = 8
B_PER_CORE = BATCH // CORES
N_UP = 8              # u' = kern*4 + mc, bins 0..511 in 4 chunks per kern
EXT = HOP * BT_COLS + 1537  # zero-extended xpad length for rev strides

_cache = {}


def _build_device_kernel(whoist=False, ot_joint=True, obufs=4, psbufs=4,
                         out_eng="sync", in_eng="sync", w_eng="scalar",
                         out_split=True, order="pipelined", **_ignored):
    import concourse.bacc as bacc
    import concourse.mybir as mybir
    from concourse import tile

    nc = bacc.Bacc("TRN2", target_bir_lowering=False, debug=False,
                   num_devices=CORES)
    f32 = mybir.dt.float32
    f32r = mybir.dt.float32r

    # xin: 4 parity-packed signal arrays per batch:
    #   src 0: bte[jj,e,m] = xpad[512m + 256e + 2jj]
    #   src 1: rve[jj,e,m] = xpad[512m + 1536 - 256e - 2jj]
    #   src 2: bto[jj,e,m] = xpad[512m + 256e + 2jj + 1]
    #   src 3: rvo[jj,e,m] = xpad[512m + 1535 - 256e - 2jj]
    xin_d = nc.dram_tensor("xin", [B_PER_CORE, 4, 128, 2, BT_COLS], f32r,
                          kind="ExternalInput")
    # w[u', jj, par, c, mm]: folded parity weights for bins < 512
    w_d = nc.dram_tensor("w", [N_UP, 128, 2, 4, 128], f32r,
                         kind="ExternalInput")
    # o[u', mm, b*1028 + half*514 + f]: half 0 = E, 1 = O
    o_d = nc.dram_tensor("o", [N_UP, 128, B_PER_CORE * 2 * PAD_FRAMES],
                         f32, kind="ExternalOutput")

    with tile.TileContext(nc) as tc:
        with (
            tc.tile_pool(name="inp", bufs=1) as inp,
            tc.tile_pool(name="zp", bufs=1) as zpool,
            tc.tile_pool(name="wpool", bufs=8) as wpool,
            tc.tile_pool(name="op", bufs=obufs) as op,
            tc.tile_pool(name="psp", bufs=psbufs, space="PSUM") as psp,
        ):
            ins = [[None] * 4 for _ in range(B_PER_CORE)]
            # z[par][s][b][c]: folded frames; par 0 = even, 1 = odd;
            # s 0 = plus (cos), 1 = minus (sin)
            zt = [[[[None] * 4 for _ in range(B_PER_CORE)]
                   for _ in range(2)] for _ in range(2)]
            for b in range(B_PER_CORE):
                for src in range(4):
                    ins[b][src] = inp.tile([128, 2, BT_COLS], f32r,
                                           name=f"in{b}{src}",
                                           tag=f"in{b}{src}")
                for par in range(2):
                    for s in range(2):
                        for c in range(4):
                            zt[par][s][b][c] = zpool.tile(
                                [128, PAD_FRAMES], f32r,
                                name=f"z{par}{s}{b}{c}",
                                tag=f"z{par}{s}{b}{c}")

            def fold(b, s, lo, hi):
                # sin-side folds ride the idle GpSimd engine so the DVE
                # stream (PSUM copies) never blocks behind them
                eng = nc.vector if s == 0 else nc.gpsimd
                dve_op = (eng.tensor_add, eng.tensor_sub)[s]
                for par in range(2):
                    bt_t, rv_t = ins[b][2 * par], ins[b][2 * par + 1]
                    for c in range(4):
                        sh, rh = c // 2, 1 - c // 2
                        dve_op(out=zt[par][s][b][c][:, lo:hi],
                               in0=bt_t[:, c % 2, lo + sh:hi + sh],
                               in1=rv_t[:, c % 2, lo + rh:hi + rh])
                if s == 0:
                    # even lane (c=0, jj=0) is n=0: win[0] = 0 frees its
                    # weight slot for the cos n=1024 column; z+E lane 0
                    # must hold y_f[1024] = bte[0, 0, m+2].
                    nc.vector.tensor_copy(
                        out=zt[0][0][b][0][0:1, lo:hi],
                        in_=ins[b][0][0:1, 0, lo + 2:hi + 2])

            in_q = {"sync": nc.sync, "scalar": nc.scalar}[in_eng]
            w_q = {"sync": nc.sync, "scalar": nc.scalar}[w_eng]
            # Head scheduling: first matmul group needs w[0] + first-half
            # b0 inputs + the z+ folds of that half.
            SPLIT = 264
            MID = N_GROUPS[1][0]
            wts = []
            for up in range(N_UP):
                wts.append(wpool.tile([128, 2, 4, 128], f32r,
                                      name=f"wt{up}", tag="wt"))
            w_q.dma_start(out=wts[0][:, 0], in_=w_d[0, :, 0])
            for src in range(2):
                in_q.dma_start(out=ins[0][src][:, :, :SPLIT],
                                  in_=xin_d[0, src, :, :, :SPLIT])
            w_q.dma_start(out=wts[0][:, 1], in_=w_d[0, :, 1])
            for src in range(2, 4):
                in_q.dma_start(out=ins[0][src][:, :, :SPLIT],
                                  in_=xin_d[0, src, :, :, :SPLIT])
            fold(0, 0, 0, MID)
            w_q.dma_start(out=wts[1], in_=w_d[1])
            for src in range(4):
                in_q.dma_start(out=ins[0][src][:, :, SPLIT:],
                                  in_=xin_d[0, src, :, :, SPLIT:])
            fold(0, 0, MID, PAD_FRAMES)
            for src in range(4):
                in_q.dma_start(out=ins[1][src][:, :, :SPLIT],
                                  in_=xin_d[1, src, :, :, :SPLIT])
            fold(1, 0, 0, MID)
            w_q.dma_start(out=wts[2], in_=w_d[2])
            for src in range(4):
                in_q.dma_start(out=ins[1][src][:, :, SPLIT:],
                                  in_=xin_d[1, src, :, :, SPLIT:])
            fold(1, 0, MID, PAD_FRAMES)
            fold(0, 1, 0, PAD_FRAMES)
            fold(1, 1, 0, PAD_FRAMES)
            if whoist:
                # weights up front: keeps the ACT queue DMA-only
                for up in range(3, N_UP):
                    w_q.dma_start(out=wts[up], in_=w_d[up])

            if order == "pipelined":
                # front-load b0 units while b1 inputs/folds stream in
                sched = ([(0, 0), (1, 0), (2, 0), (3, 0),
                          (0, 1), (1, 1), (2, 1), (3, 1)]
                         + [(up, b) for up in range(4, N_UP)
                            for b in range(B_PER_CORE)])
            else:
                sched = [(up, b) for up in range(N_UP)
                         for b in range(B_PER_CORE)]
            emitted_w = set()
            otj_map = {}
            out_q = {"gpsimd": nc.gpsimd, "sync": nc.sync,
                     "scalar": nc.scalar}[out_eng]
            for up, b in sched:
                kern = up // 4
                wt = wts[up]
                if not whoist and up >= 3 and up not in emitted_w:
                    emitted_w.add(up)
                    w_q.dma_start(out=wt, in_=w_d[up])
                if ot_joint:
                    if up not in otj_map:
                        otj_map[up] = op.tile(
                            [128, B_PER_CORE * 2 * PAD_FRAMES], f32,
                            name="otj", tag="ot")
                    ot = otj_map[up][:, b * 2 * PAD_FRAMES:
                                     (b + 1) * 2 * PAD_FRAMES]
                else:
                    ot = op.tile([128, 2 * PAD_FRAMES], f32,
                                 name="ot", tag="ot")
                for f0, ng in N_GROUPS:
                    psE = psp.tile([128, ng], f32, name="psE", tag="psE")
                    psO = psp.tile([128, ng], f32, name="psO", tag="psO")
                    for c in range(4):
                        nc.tensor.matmul(
                            psE, wt[:, 0, c, :],
                            zt[0][kern][b][c][:, f0:f0 + ng],
                            start=(c == 0), stop=(c == 3))
                    for c in range(4):
                        nc.tensor.matmul(
                            psO, wt[:, 1, c, :],
                            zt[1][kern][b][c][:, f0:f0 + ng],
                            start=(c == 0), stop=(c == 3))
                    nc.vector.tensor_copy(
                        out=ot[:, f0:f0 + ng], in_=psE)
                    # ACT is otherwise idle; halves the DVE copy load
                    nc.scalar.copy(
                        out=ot[:, PAD_FRAMES + f0:PAD_FRAMES + f0 + ng],
                        in_=psO)
                base = b * 2 * PAD_FRAMES
                if out_split:
                    out_q.dma_start(
                        out=o_d[up, :, base:base + PAD_FRAMES],
                        in_=ot[:, :PAD_FRAMES])
                    out_q.dma_start(
                        out=o_d[up, :, base + PAD_FRAMES:
                                base + 2 * PAD_FRAMES],
                        in_=ot[:, PAD_FRAMES:])
                else:
                    out_q.dma_start(
                        out=o_d[up, :, base:base + 2 * PAD_FRAMES],
                        in_=ot)
    nc.compile()
    return nc


def _get_nc():
    if "nc" not in _cache:
        _cache["nc"] = _build_device_kernel()
    return _cache["nc"]


def _host_prep(x, wsin, wcos):
    from numpy.lib.stride_tricks import as_strided

    x = np.asarray(x, dtype=np.float32)
    wsin = np.asarray(wsin, dtype=np.float32).reshape(N_FFT, N_FFT)
    wcos = np.asarray(wcos, dtype=np.float32).reshape(N_FFT, N_FFT)

    xpad = np.pad(x, ((0, 0), (N_FFT // 2, N_FFT // 2)), mode="reflect")
    xe = np.zeros((BATCH, EXT), np.float32)
    xe[:, :xpad.shape[1]] = xpad
    sb = xe.strides[1]
    s0 = xe.strides[0]

    xin = np.empty((BATCH, 4, 128, 2, BT_COLS), np.float32)
    shape = (BATCH, 128, 2, BT_COLS)
    xin[:, 0] = as_strided(xe, shape, (s0, 2 * sb, 256 * sb, 512 * sb))
    xin[:, 2] = as_strided(xe[:, 1:], shape,
                           (s0, 2 * sb, 256 * sb, 512 * sb))
    xin[:, 1] = as_strided(xe[:, 1536:], shape,
                           (s0, -2 * sb, -256 * sb, 512 * sb))
    xin[:, 3] = as_strided(xe[:, 1535:], shape,
                           (s0, -2 * sb, -256 * sb, 512 * sb))

    # folded parity weights for bin rows k < 512
    wf = np.empty((N_UP, 128, 2, 4, 128), np.float32)
    jj = np.arange(128)
    for kern, wm in enumerate((wcos, -wsin)):
        for mc in range(4):
            rows = wm[128 * mc:128 * mc + 128]       # (128 bins, 2048)
            for c in range(4):
                n_ev = 256 * c + 2 * jj
                wf[kern * 4 + mc, :, 0, c, :] = rows[:, n_ev].T
                wf[kern * 4 + mc, :, 1, c, :] = rows[:, n_ev + 1].T
    # n=0 even lane is dead (win[0] = 0): carry the cos n=1024 column
    wf[0:4, 0, 0, 0, :] = wcos[:512, 1024].reshape(4, 128)

    # host bin-512 rows (not representable in the parity fold)
    fr = np.lib.stride_tricks.sliding_window_view(
        xpad, N_FFT, axis=1)[:, ::HOP]               # (B, 513, 2048)
    row512 = np.empty((2, BATCH, FRAMES), np.float32)
    for kern, wm in enumerate((wcos, -wsin)):
        row512[kern] = np.einsum('bfn,n->bf', fr, wm[512],
                                 optimize=True).astype(np.float32)
    return xin, wf, row512


def _host_assemble(outs, row512):
    # outs: 8 arrays (8, 128, 2*2*514); E/O halves per batch
    per_batch_E, per_batch_O = [], []
    for o in outs:
        for b in range(B_PER_CORE):
            base = b * 2 * PAD_FRAMES
            per_batch_E.append(o[:, :, base:base + FRAMES])
            per_batch_O.append(
                o[:, :, base + PAD_FRAMES:base + PAD_FRAMES + FRAMES])
    E = np.stack(per_batch_E).reshape(BATCH, 2, 512, FRAMES)
    O = np.stack(per_batch_O).reshape(BATCH, 2, 512, FRAMES)

    outs_full = []
    for kern, msign in ((0, 1.0), (1, -1.0)):
        lo = E[:, kern] + O[:, kern]               # bins 0..511
        hi = E[:, kern] - O[:, kern]               # bins 1024-k
        if kern == 1:
            hi = -hi
        head = np.concatenate(
            [lo, row512[kern][:, None, :], hi[:, 511:0:-1], hi[:, 0:1]],
            axis=1)                                 # bins 0..1024
        full = np.concatenate([head, msign * head[:, 1023:0:-1]], axis=1)
        outs_full.append(np.ascontiguousarray(full, dtype=np.float32))
    return tuple(outs_full)


def kernel(x, wsin, wcos):
    from concourse.bass_utils import run_bass_kernel_spmd

    nc = _get_nc()
    xin, wf, row512 = _host_prep(x, wsin, wcos)
    in_maps = [
        {"xin": xin[i * B_PER_CORE:(i + 1) * B_PER_CORE], "w": wf}
        for i in range(CORES)
    ]
    res = run_bass_kernel_spmd(nc, in_maps, core_ids=list(range(CORES)))
    return _host_assemble(
        [res.results[i]["o"] for i in range(CORES)], row512)
